# revision 8
# baseline (speedup 1.0000x reference)
"""Bidirectional cross-patch attention on 8 trn2 NeuronCores.

Sharding: data-parallel over B (4 batches x 2 cores), head-parallel within
each batch pair (6 heads per core). Each core computes q/k/v projections for
its heads, per-head masked attention, and a partial output projection; the
host sums the two partials per batch and adds the output bias.

Mask handling: allowed[i,j] = ctx_i ? ctx_j : 1. The additive -1e30 mask is
rank-1 (u_i * w_j with u=ctx, w=-1e30*(1-ctx)), so it is fused into the QK^T
matmul as a 65th contraction row. Logits are bounded (|s|~few), so softmax
needs no max subtraction: P = exp(scale*S_masked), denominator from an extra
ones-column in V.
"""

import numpy as np
import ml_dtypes

import concourse.bass as bass
import concourse.mybir as mybir
import concourse.tile as tile
from concourse.bass_utils import run_bass_kernel_spmd

BF16 = mybir.dt.bfloat16
F32 = mybir.dt.float32
bf16 = ml_dtypes.bfloat16

B, K, D, H, HD = 4, 2048, 768, 12, 64
HPC = 6        # heads per core
NPAIR = 3      # head pairs per core
NCHUNK = 6     # 768 / 128 contraction chunks
N_CORES = 8
NEG = -1e30
SCALE = 0.125  # 1/sqrt(HD)
NT = K // 128  # 16 token tiles of 128
NIB = K // 512  # 4 i-blocks of 512


def _split_multi_waits(nc, max_waits=1):
    """Walrus (CoreV3) rejects >1 sync-wait on one instruction; move extras
    onto no-op instructions inserted just before, preserving engine order."""
    for f in nc.m.functions:
        for bb in f.blocks:
            new_insts = []
            for inst in bb.instructions:
                si = inst.sync_info
                if si is not None and si.on_wait and len(si.on_wait) > max_waits:
                    waits = list(si.on_wait)
                    extra, keep = waits[:-max_waits], waits[-max_waits:]
                    for i in range(0, len(extra), max_waits):
                        chunk = extra[i:i + max_waits]
                        nop = mybir.InstNoOp(
                            name=f"waitsplit-{inst.name}-{i}",
                            engine=inst.engine,
                            sync_info=mybir.SyncInfo(on_wait=chunk, on_update=[]),
                        )
                        new_insts.append(nop)
                    si.on_wait = keep
                new_insts.append(inst)
            bb.instructions[:] = new_insts


def _build_nc():
    nc = bass.Bass()
    xT = nc.dram_tensor("xT", [NCHUNK, 128, K], BF16, kind="ExternalInput")
    wqT = nc.dram_tensor("wqT", [NCHUNK, 128, 384], BF16, kind="ExternalInput")
    wkT = nc.dram_tensor("wkT", [NCHUNK, 128, 384], BF16, kind="ExternalInput")
    wvT = nc.dram_tensor("wvT", [NCHUNK, 128, 384], BF16, kind="ExternalInput")
    woT = nc.dram_tensor("woT", [NPAIR, 128, D], BF16, kind="ExternalInput")
    bqv = nc.dram_tensor("bqv", [128, NPAIR], F32, kind="ExternalInput")
    bkv = nc.dram_tensor("bkv", [128, NPAIR], F32, kind="ExternalInput")
    bvv = nc.dram_tensor("bvv", [1, 384], BF16, kind="ExternalInput")
    uv = nc.dram_tensor("uv", [1, K], BF16, kind="ExternalInput")
    wv = nc.dram_tensor("wv", [1, K], BF16, kind="ExternalInput")
    out = nc.dram_tensor("out", [K, D], F32, kind="ExternalOutput")

    with tile.TileContext(nc) as tc:
        with (
            tc.tile_pool(name="const", bufs=1) as constp,
            tc.tile_pool(name="qpair", bufs=2) as qpp,
            tc.tile_pool(name="heads", bufs=2) as qkh,
            tc.tile_pool(name="vpool", bufs=2) as vpl,
            tc.tile_pool(name="ptp", bufs=17) as ptp,
            tc.tile_pool(name="yhp", bufs=2) as yhp,
            tc.tile_pool(name="ypk", bufs=1) as ypp,
            tc.tile_pool(name="small", bufs=2) as smp,
            tc.tile_pool(name="ost", bufs=2) as osp,
            tc.tile_pool(name="dscr", bufs=4, space="DRAM") as dsp,
            tc.tile_pool(name="ps_s", bufs=2, space="PSUM") as ps_s,
            tc.tile_pool(name="ps_y", bufs=2, space="PSUM") as ps_y,
            tc.tile_pool(name="ps_p", bufs=2, space="PSUM") as ps_p,
        ):
            # ---- load persistent operands
            xts = []
            for c in range(NCHUNK):
                t = constp.tile([128, K], BF16, tag=f"xt{c}")
                nc.sync.dma_start(out=t, in_=xT[c])
                xts.append(t)

            def load_w(name, dram, n, cols):
                ts = []
                for c in range(n):
                    t = constp.tile([128, cols], BF16, tag=f"{name}{c}")
                    nc.sync.dma_start(out=t, in_=dram[c])
                    ts.append(t)
                return ts

            wqs = load_w("wq", wqT, NCHUNK, 384)
            wks = load_w("wk", wkT, NCHUNK, 384)
            wvs = load_w("wv", wvT, NCHUNK, 384)
            wos = load_w("wo", woT, NPAIR, D)

            bq_sb = constp.tile([128, NPAIR], F32, tag="bq")
            nc.sync.dma_start(out=bq_sb, in_=bqv[:])
            bk_sb = constp.tile([128, NPAIR], F32, tag="bk")
            nc.sync.dma_start(out=bk_sb, in_=bkv[:])
            bv_sb = constp.tile([1, 384], BF16, tag="bv")
            nc.sync.dma_start(out=bv_sb, in_=bvv[:])
            ones_sb = constp.tile([1, 128], BF16, tag="ones")
            nc.vector.memset(ones_sb, 1.0)

            ypk = [
                ypp.tile([128, K], BF16, tag=f"ypk{c}", name=f"ypk{c}")
                for c in range(NPAIR)
            ]

            for p in range(NPAIR):
                hsl = slice(p * 128, (p + 1) * 128)
                # ---- q^T / k^T projection for this head pair: [128(dq), K]
                pair_tiles = {}
                for nm, ws, b_sb in (("q", wqs, bq_sb), ("k", wks, bk_sb)):
                    tp = qpp.tile([128, K], BF16, tag=f"{nm}pair")
                    for ib in range(NIB):
                        isl = slice(ib * 512, (ib + 1) * 512)
                        ps = ps_p.tile([128, 512], F32, tag="proj")
                        for c in range(NCHUNK):
                            nc.tensor.matmul(
                                ps, ws[c][:, hsl], xts[c][:, isl],
                                start=(c == 0), stop=(c == NCHUNK - 1),
                            )
                        nc.vector.tensor_scalar_add(tp[:, isl], ps, b_sb[:, p:p + 1])
                    pair_tiles[nm] = tp
                # per-head 65-row tiles: rows 0..63 head data, row 64 = mask row
                qh, kh = [], []
                for hh in range(2):
                    qt = qkh.tile([65, K], BF16, tag=f"qh{hh}")
                    kt = qkh.tile([65, K], BF16, tag=f"kh{hh}")
                    nc.gpsimd.dma_start(out=qt[0:64, :], in_=pair_tiles["q"][hh * 64:(hh + 1) * 64, :])
                    nc.gpsimd.dma_start(out=kt[0:64, :], in_=pair_tiles["k"][hh * 64:(hh + 1) * 64, :])
                    nc.gpsimd.dma_start(out=qt[64:65, :], in_=uv[:])
                    nc.gpsimd.dma_start(out=kt[64:65, :], in_=wv[:])
                    qh.append(qt)
                    kh.append(kt)
                # ---- v projection: natural layout [t, dv], packed per (tile, head)
                vh = vpl.tile([128, NT, 2, 65], BF16, tag="vh")
                nc.vector.memset(vh[:, :, :, 64:65], 1.0)
                for tt in range(NT):
                    tsl = slice(tt * 128, (tt + 1) * 128)
                    ps = ps_p.tile([128, 512], F32, tag="proj")
                    for c in range(NCHUNK):
                        nc.tensor.matmul(
                            ps[:, 0:128], xts[c][:, tsl], wvs[c][:, hsl],
                            start=(c == 0), stop=False,
                        )
                    nc.tensor.matmul(
                        ps[:, 0:128], ones_sb[:, 0:128], bv_sb[:, hsl],
                        start=False, stop=True,
                    )
                    for hh in range(2):
                        nc.vector.tensor_copy(vh[:, tt, hh, 0:64], ps[:, hh * 64:(hh + 1) * 64])

                # ---- attention per head
                for hh in range(2):
                    pts = []
                    for jc in range(NT):
                        ptile = ptp.tile([128, K], BF16, tag="pt")
                        for ih in range(2):
                            hsl2 = slice(ih * 1024, (ih + 1) * 1024)
                            s_ps = ps_s.tile([128, 1024], F32, tag="s")
                            for ib in range(2):
                                isl = slice(ib * 512, (ib + 1) * 512)
                                nc.tensor.matmul(
                                    s_ps[:, isl],
                                    kh[hh][:, jc * 128:(jc + 1) * 128],
                                    qh[hh][:, ih * 1024 + ib * 512:ih * 1024 + (ib + 1) * 512],
                                    start=True, stop=True,
                                )
                            nc.scalar.activation(
                                ptile[:, hsl2], s_ps,
                                mybir.ActivationFunctionType.Exp, scale=SCALE,
                            )
                        pts.append(ptile)
                    yht = yhp.tile([64, K], BF16, tag="yh")
                    for ib in range(NIB):
                        isl = slice(ib * 512, (ib + 1) * 512)
                        y_ps = ps_y.tile([65, 512], F32, tag="y")
                        for jc in range(NT):
                            nc.tensor.matmul(
                                y_ps, vh[:, jc, hh, :], pts[jc][:, isl],
                                start=(jc == 0), stop=(jc == NT - 1),
                            )
                        # normalize: r = 1/sum row, broadcast across partitions
                        rt = smp.tile([65, 512], F32, tag="r")
                        nc.vector.reciprocal(rt[64:65, :], y_ps[64:65, :])
                        dscr = dsp.tile([1, 512], F32, tag="ds")
                        nc.gpsimd.dma_start(out=dscr, in_=rt[64:65, :])
                        rb = smp.tile([64, 512], F32, tag="rb")
                        bcast = bass.AP(
                            tensor=dscr.tensor, offset=dscr.offset,
                            ap=[[0, 64], list(dscr.ap[-1])],
                        )
                        nc.gpsimd.dma_start(out=rb, in_=bcast)
                        nc.vector.tensor_mul(yht[:, isl], y_ps[0:64, :], rb)
                    # repack into [128(dy), K] via DMA (partition shift)
                    nc.gpsimd.dma_start(out=ypk[p][hh * 64:(hh + 1) * 64, :], in_=yht)

            # ---- output projection (partial over this core's 384 dy)
            for tt in range(NT):
                tsl = slice(tt * 128, (tt + 1) * 128)
                ot = osp.tile([128, D], F32, tag="ost")
                for oc, osz in ((0, 512), (1, 256)):
                    ps = ps_p.tile([128, 512], F32, tag="proj")
                    for c in range(NPAIR):
                        nc.tensor.matmul(
                            ps[:, 0:osz],
                            ypk[c][:, tsl],
                            wos[c][:, oc * 512:oc * 512 + osz],
                            start=(c == 0), stop=(c == NPAIR - 1),
                        )
                    nc.vector.tensor_copy(ot[:, oc * 512:oc * 512 + osz], ps[:, 0:osz])
                nc.sync.dma_start(out=out[tsl, :], in_=ot)

    _split_multi_waits(nc)
    return nc


_NC = None


def _get_nc():
    global _NC
    if _NC is None:
        _NC = _build_nc()
    return _NC


def make_in_maps(x, is_context, Wq, bq, Wk, bk, Wv, bv, Wo):
    """Host-side sharding/layout prep: per-core input dicts."""
    in_maps = []
    xTb = {}
    uvb = {}
    wvb = {}
    for b in range(B):
        ctx = is_context[b].astype(np.float32)
        xTb[b] = np.ascontiguousarray(x[b].T).astype(bf16).reshape(NCHUNK, 128, K)
        uvb[b] = ctx.reshape(1, K).astype(bf16)
        wvb[b] = (NEG * (1.0 - ctx)).reshape(1, K).astype(bf16)
    WoT = np.ascontiguousarray(Wo.T)  # [dy, dout]
    for core in range(N_CORES):
        b = core // 2
        half = core % 2
        sel = slice(384 * half, 384 * (half + 1))
        in_maps.append({
            "xT": xTb[b],
            "wqT": np.ascontiguousarray(Wq[sel].T).astype(bf16).reshape(NCHUNK, 128, 384),
            "wkT": np.ascontiguousarray(Wk[sel].T).astype(bf16).reshape(NCHUNK, 128, 384),
            "wvT": np.ascontiguousarray(Wv[sel].T).astype(bf16).reshape(NCHUNK, 128, 384),
            "woT": WoT[sel].astype(bf16).reshape(NPAIR, 128, D),
            "bqv": np.ascontiguousarray(bq[sel].reshape(NPAIR, 128).T).astype(np.float32),
            "bkv": np.ascontiguousarray(bk[sel].reshape(NPAIR, 128).T).astype(np.float32),
            "bvv": bv[sel].reshape(1, 384).astype(bf16),
            "uv": uvb[b],
            "wv": wvb[b],
        })
    return in_maps


def combine_results(results, bo):
    out = np.zeros((B, K, D), np.float32)
    for b in range(B):
        out[b] = results[2 * b]["out"] + results[2 * b + 1]["out"] + bo
    return out


def kernel(x, is_context, Wq, bq, Wk, bk, Wv, bv, Wo, bo):
    x = np.asarray(x, np.float32)
    is_context = np.asarray(is_context)
    args = [np.asarray(a, np.float32) for a in (Wq, bq, Wk, bk, Wv, bv, Wo)]
    nc = _get_nc()
    in_maps = make_in_maps(x, is_context, *args)
    res = run_bass_kernel_spmd(nc, in_maps, list(range(N_CORES)), trace=False)
    return combine_results(res.results, np.asarray(bo, np.float32))


# revision 51
# speedup vs baseline: 2.4905x; 2.4905x over previous
"""Bidirectional cross-patch attention on 8 trn2 NeuronCores.

Sharding: data-parallel over B (4 batches x 2 cores), head-parallel within
each batch pair (6 heads per core). Each core computes q/k/v projections for
its heads, per-head masked attention, and a partial output projection; the
host sums the two partials per batch and adds the output bias.

Mask handling: allowed[i,j] = ctx_i ? ctx_j : 1. The additive -1e30 mask is
rank-1 (u_i * w_j with u=ctx, w=-1e30*(1-ctx)), so it is fused into the QK^T
matmul as a 65th contraction row. Logits are bounded (|s|~few), so softmax
needs no max subtraction: P = exp(scale*S_masked), denominator from an extra
ones-column in V.

Schedule: per head, QK tiles (S^T in PSUM, [128,1024]) ping-pong with ACT exp
(the pacing engine); AV accumulates per j-chunk right after its exp so P^T
tiles free early. Projections of the NEXT head-pair are emitted interleaved
into the exp gaps to keep TensorE dense (HAM warm).
"""

from collections import deque

import numpy as np
import ml_dtypes

import concourse.bass as bass
import concourse.mybir as mybir
import concourse.tile as tile
from concourse.bass_utils import run_bass_kernel_spmd

BF16 = mybir.dt.bfloat16
F32 = mybir.dt.float32
bf16 = ml_dtypes.bfloat16

B, K, D, H, HD = 4, 2048, 768, 12, 64
HPC = 6        # heads per core
NPAIR = 3      # head pairs per core
NCHUNK = 6     # 768 / 128 contraction chunks
N_CORES = 8
NEG = -1e30
SCALE = 0.125  # 1/sqrt(HD)
NT = K // 128   # 16 token tiles of 128
NIB = K // 512  # 4 i-blocks of 512


def _split_multi_waits(nc, max_waits=1):
    """Walrus (CoreV3) rejects >1 sync-wait on one instruction; move extras
    onto no-op instructions inserted just before, preserving engine order."""
    for f in nc.m.functions:
        for bb in f.blocks:
            new_insts = []
            for inst in bb.instructions:
                si = inst.sync_info
                if si is not None and si.on_wait and len(si.on_wait) > max_waits:
                    waits = list(si.on_wait)
                    extra, keep = waits[:-max_waits], waits[-max_waits:]
                    for i in range(0, len(extra), max_waits):
                        chunk = extra[i:i + max_waits]
                        nop = mybir.InstNoOp(
                            name=f"waitsplit-{inst.name}-{i}",
                            engine=inst.engine,
                            sync_info=mybir.SyncInfo(on_wait=chunk, on_update=[]),
                        )
                        new_insts.append(nop)
                    si.on_wait = keep
                new_insts.append(inst)
            bb.instructions[:] = new_insts


def _build_nc(skip=True, split_waits=True):
    nc = bass.Bass()
    xT = nc.dram_tensor("xT", [NCHUNK, 128, K], BF16, kind="ExternalInput")
    wqT = nc.dram_tensor("wqT", [NCHUNK, 128, 384], BF16, kind="ExternalInput")
    wkT = nc.dram_tensor("wkT", [NCHUNK, 128, 384], BF16, kind="ExternalInput")
    wvT = nc.dram_tensor("wvT", [NCHUNK, 128, 390], BF16, kind="ExternalInput")
    woT = nc.dram_tensor("woT", [NPAIR, 128, D], BF16, kind="ExternalInput")
    bqv = nc.dram_tensor("bqv", [128, NPAIR], F32, kind="ExternalInput")
    bkv = nc.dram_tensor("bkv", [128, NPAIR], F32, kind="ExternalInput")
    bvv = nc.dram_tensor("bvv", [1, 390], BF16, kind="ExternalInput")
    uv = nc.dram_tensor("uv", [1, K], BF16, kind="ExternalInput")
    wv = nc.dram_tensor("wv", [1, K], BF16, kind="ExternalInput")
    idm = nc.dram_tensor("idm", [128, 128], F32, kind="ExternalInput")
    out = nc.dram_tensor("out", [K, D], F32, kind="ExternalOutput")

    with tile.TileContext(nc) as tc:
        with (
            tc.tile_pool(name="const", bufs=1) as constp,
            tc.tile_pool(name="qpair", bufs=2) as qpp,
            tc.tile_pool(name="heads", bufs=2) as qkh,
            tc.tile_pool(name="vpool", bufs=2) as vpl,
            tc.tile_pool(name="ptp", bufs=8) as ptp,
            tc.tile_pool(name="yhp", bufs=2) as yhp,
            tc.tile_pool(name="ypk", bufs=1) as ypp,
            tc.tile_pool(name="small", bufs=2) as smp,
            tc.tile_pool(name="ost", bufs=2) as osp,
            tc.tile_pool(name="ps_a", bufs=2, space="PSUM") as ps_a,
            tc.tile_pool(name="ps_y", bufs=2, space="PSUM") as ps_y,
            tc.tile_pool(name="ps_t", bufs=1, space="PSUM") as ps_t,
        ):
            # ---- load persistent operands
            xts = []
            for c in range(NCHUNK):
                t = constp.tile([128, K], BF16, tag=f"xt{c}", name=f"xt{c}")
                nc.sync.dma_start(out=t, in_=xT[c])
                xts.append(t)

            def load_w(name, dram, n, cols):
                ts = []
                for c in range(n):
                    t = constp.tile([128, cols], BF16, tag=f"{name}{c}",
                                    name=f"{name}{c}")
                    nc.sync.dma_start(out=t, in_=dram[c])
                    ts.append(t)
                return ts

            wqs = load_w("wq", wqT, NCHUNK, 384)
            wks = load_w("wk", wkT, NCHUNK, 384)

            bq_sb = constp.tile([128, NPAIR], F32, tag="bq")
            nc.sync.dma_start(out=bq_sb, in_=bqv[:])
            bk_sb = constp.tile([128, NPAIR], F32, tag="bk")
            nc.sync.dma_start(out=bk_sb, in_=bkv[:])
            ones_sb = constp.tile([1, 128], BF16, tag="ones")
            nc.vector.memset(ones_sb, 1.0)
            one_f32 = constp.tile([1, 1], F32, tag="one32")
            nc.vector.memset(one_f32, 1.0)
            ones64 = constp.tile([1, 64], F32, tag="ones64")
            nc.vector.memset(ones64, 1.0)
            # later-needed operands: allocate now, DMA after pair-0 q/k proj
            wvs = [constp.tile([128, 390], BF16, tag=f"wv{c}", name=f"wv{c}")
                   for c in range(NCHUNK)]
            wos = [constp.tile([128, D], BF16, tag=f"wo{c}", name=f"wo{c}")
                   for c in range(NPAIR)]
            bv_sb = constp.tile([1, 390], BF16, tag="bv")
            id_sb = constp.tile([128, 128], F32, tag="idm")

            def load_deferred():
                for c in range(NCHUNK):
                    nc.sync.dma_start(out=wvs[c], in_=wvT[c])
                nc.sync.dma_start(out=bv_sb, in_=bvv[:])
                nc.sync.dma_start(out=id_sb, in_=idm[:])
                for c in range(NPAIR):
                    nc.sync.dma_start(out=wos[c], in_=woT[c])

            ypk = [
                ypp.tile([128, K], BF16, tag=f"ypk{c}", name=f"ypk{c}")
                for c in range(NPAIR)
            ]

            def make_pair_setup(p):
                """Allocate pair-p tiles; return (state, qk_emitters, v_emitters)."""
                hsl = slice(p * 128, (p + 1) * 128)
                st = {
                    "qpair": qpp.tile([128, K], BF16, tag="qpair", name=f"qp{p}"),
                    "kpair": qpp.tile([128, K], BF16, tag="kpair", name=f"kp{p}"),
                    "qh": [qkh.tile([65, K], BF16, tag=f"qh{hh}", name=f"q{p}h{hh}")
                           for hh in range(2)],
                    "kh": [qkh.tile([65, K], BF16, tag=f"kh{hh}", name=f"k{p}h{hh}")
                           for hh in range(2)],
                    "vh": vpl.tile([128, NT, 2, 65], BF16, tag="vh", name=f"v{p}"),
                }
                ems = []

                def qk_group(nm, ws, b_sb, tp, ib):
                    def em():
                        isl = slice(ib * 512, (ib + 1) * 512)
                        ps = ps_a.tile([128, 1024], F32, tag="a", name=f"pj{p}{nm}{ib}")
                        for c in range(NCHUNK):
                            nc.tensor.matmul(
                                ps[:, 0:512], ws[c][:, hsl], xts[c][:, isl],
                                start=(c == 0), stop=(c == NCHUNK - 1),
                            )
                        nc.vector.tensor_scalar_add(tp[:, isl], ps[:, 0:512],
                                                    b_sb[:, p:p + 1])
                    return em

                def repack(cs):
                    # per column-half, so the first QK (reading cols [0:1024))
                    # starts before the whole projection lands
                    def em():
                        for hh in range(2):
                            nc.sync.dma_start(
                                out=st["qh"][hh][0:64, cs],
                                in_=st["qpair"][hh * 64:(hh + 1) * 64, cs])
                            nc.sync.dma_start(
                                out=st["kh"][hh][0:64, cs],
                                in_=st["kpair"][hh * 64:(hh + 1) * 64, cs])
                            if cs.start == 0:
                                nc.sync.dma_start(out=st["qh"][hh][64:65, :],
                                                  in_=uv[:])
                                nc.sync.dma_start(out=st["kh"][hh][64:65, :],
                                                  in_=wv[:])
                    return em

                for ib in range(2):
                    ems.append(qk_group("q", wqs, bq_sb, st["qpair"], ib))
                for ib in range(2):
                    ems.append(qk_group("k", wks, bk_sb, st["kpair"], ib))
                ems.append(repack(slice(0, 1024)))
                for ib in range(2, NIB):
                    ems.append(qk_group("q", wqs, bq_sb, st["qpair"], ib))
                for ib in range(2, NIB):
                    ems.append(qk_group("k", wks, bk_sb, st["kpair"], ib))
                ems.append(repack(slice(1024, 2048)))

                vems = []
                # wvT/bvv columns are host-extended per pair to
                # [h0 64 | 0-col, h1 64 | 0-col] with bias 1.0 on the 0-cols,
                # so v comes out of PSUM as [v|1] per head, contiguously.
                vsl = slice(p * 130, (p + 1) * 130)

                def v_group(tt):
                    def em():
                        tsl = slice(tt * 128, (tt + 1) * 128)
                        ps = ps_a.tile([128, 1024], F32, tag="a", name=f"pv{p}{tt}")
                        for c in range(NCHUNK):
                            nc.tensor.matmul(
                                ps[:, 0:130], xts[c][:, tsl], wvs[c][:, vsl],
                                start=(c == 0), stop=False,
                            )
                        nc.tensor.matmul(
                            ps[:, 0:130], ones_sb[:, 0:128], bv_sb[:, vsl],
                            start=False, stop=True,
                        )
                        nc.vector.tensor_copy(
                            st["vh"][:, tt].rearrange("p a b -> p (a b)"),
                            ps[:, 0:130])
                    return em

                for tt in range(NT):
                    vems.append(v_group(tt))
                return st, ems, vems

            AV_DELAY = 7
            NH = K // 1024  # 2 i-halves per head
            JC0 = 9  # with ctx-first sorted tokens: keys j >= JC0*128 are
            # non-context and queries i < 512 are context for every batch
            # (requires 512 <= n_ctx <= JC0*128, checked on the host), so
            # S^T blocks (jc >= JC0, i < 512) are exactly masked -> skipped.

            def emit_attention_half(p, hh, ih, st, fillers, yht):
                """One head-half (1024 query columns): QK/exp/AV.

                AV matmuls are emitted AV_DELAY j-chunks late so that the
                previous half's normalize chain (which frees the y PSUM
                slots) never stalls the PE stream. Returns tail closures
                (transposed reciprocal, broadcast, muls, repack) for the
                NEXT half's filler queue.
                """
                qt, kt, vh = st["qh"][hh], st["kh"][hh], st["vh"]
                y_tiles = [
                    ps_y.tile([65, 512], F32, tag="y", name=f"y{p}{hh}{ih}{b}")
                    for b in range(2)
                ]
                av_pending = deque()

                def av(jc):
                    def em():
                        if skip and ih == 0 and jc >= JC0:
                            # only query cols [896:1024) see non-context keys;
                            # continue the b=1 accumulation group (no start:
                            # has_written persists from jc < JC0)
                            nc.tensor.matmul(
                                y_tiles[1][:, 384:512], vh[:, jc, hh, :],
                                pts[jc][:, 896:1024],
                                start=False, stop=(jc == NT - 1),
                                skip_group_check=True,
                            )
                            return
                        stop_jc = (JC0 - 1) if (skip and ih == 0) else (NT - 1)
                        for b in range(2):
                            nc.tensor.matmul(
                                y_tiles[b], vh[:, jc, hh, :],
                                pts[jc][:, b * 512:(b + 1) * 512],
                                start=(jc == 0), stop=(jc == stop_jc),
                                skip_group_check=skip and ih == 0,
                            )
                    return em

                pts = []
                for jc in range(NT):
                    skip_b0 = skip and ih == 0 and jc >= JC0
                    ptile = ptp.tile([128, 1024], BF16, tag="pt",
                                     name=f"pt{p}{hh}{ih}{jc}")
                    pts.append(ptile)
                    s_ps = ps_a.tile([128, 1024], F32, tag="a",
                                     name=f"s{p}{hh}{ih}{jc}")
                    if skip_b0:
                        # only query cols [896:1024) attend non-context keys
                        nc.tensor.matmul(
                            s_ps[:, 896:1024],
                            kt[:, jc * 128:(jc + 1) * 128],
                            qt[:, 896:1024],
                            start=True, stop=True,
                        )
                        nc.scalar.activation(
                            ptile[:, 896:1024], s_ps[:, 896:1024],
                            mybir.ActivationFunctionType.Exp, scale=SCALE,
                        )
                    else:
                        for b in range(2):
                            o = ih * 1024 + b * 512
                            nc.tensor.matmul(
                                s_ps[:, b * 512:(b + 1) * 512],
                                kt[:, jc * 128:(jc + 1) * 128],
                                qt[:, o:o + 512],
                                start=True, stop=True,
                            )
                        nc.scalar.activation(
                            ptile, s_ps,
                            mybir.ActivationFunctionType.Exp, scale=SCALE,
                        )
                    if fillers:
                        fillers.popleft()()
                    av_pending.append(av(jc))
                    if len(av_pending) > AV_DELAY:
                        av_pending.popleft()()
                while av_pending:
                    av_pending.popleft()()

                # sum rows -> SBUF (feeds the transposed reciprocal)
                s_sb = smp.tile([1, 1024], F32, tag="ssb", name=f"ss{p}{hh}{ih}")
                for b in range(2):
                    nc.vector.tensor_copy(
                        s_sb[0:1, b * 512:(b + 1) * 512],
                        y_tiles[b][64:65, :])

                rb = smp.tile([64, 1024], F32, tag="rb", name=f"rb{p}{hh}{ih}")

                def tail_recip():
                    # transpose s [1,1024] -> [128,8] via contract-1 matmuls
                    st_ps = ps_t.tile([128, 8], F32, tag="t", name=f"st{p}{hh}{ih}")
                    for tt in range(8):
                        nc.tensor.matmul(
                            st_ps[:, tt:tt + 1],
                            s_sb[0:1, tt * 128:(tt + 1) * 128],
                            one_f32[0:1, 0:1],
                            start=True, stop=True,
                        )
                    r_sb = smp.tile([128, 8], F32, tag="rsb", name=f"rr{p}{hh}{ih}")
                    nc.vector.reciprocal(r_sb, st_ps)
                    # transpose back to a row via identity matmuls
                    rrow_ps = ps_t.tile([1, 1024], F32, tag="t",
                                        name=f"rw{p}{hh}{ih}")
                    for c in range(8):
                        nc.tensor.matmul(
                            rrow_ps[0:1, c * 128:(c + 1) * 128],
                            r_sb[:, c:c + 1], id_sb,
                            start=True, stop=True,
                        )
                    rrow_sb = smp.tile([1, 1024], F32, tag="rws",
                                       name=f"rx{p}{hh}{ih}")
                    nc.vector.tensor_copy(rrow_sb, rrow_ps)
                    # replicate across 64 partitions via rank-1 matmul
                    rb_ps = ps_t.tile([64, 1024], F32, tag="t",
                                      name=f"rp{p}{hh}{ih}")
                    for c in range(2):
                        csl = slice(c * 512, (c + 1) * 512)
                        nc.tensor.matmul(rb_ps[:, csl], ones64, rrow_sb[0:1, csl],
                                         start=True, stop=True)
                    nc.vector.tensor_copy(rb, rb_ps)

                def tail_mul(b):
                    def em():
                        isl = slice(ih * 1024 + b * 512, ih * 1024 + (b + 1) * 512)
                        nc.vector.tensor_mul(yht[:, isl], y_tiles[b][0:64, :],
                                             rb[:, b * 512:(b + 1) * 512])
                    return em

                tail = deque([tail_recip] + [tail_mul(b) for b in range(2)])

                if ih == NH - 1:
                    def tail_repack():
                        nc.gpsimd.dma_start(
                            out=ypk[p][hh * 64:(hh + 1) * 64, :], in_=yht)
                    tail.append(tail_repack)
                return tail

            # pair 0 q/k projection emitted inline; its v-projection rides as
            # fillers inside the first head's QK loop (earlier ACT start)
            st, ems, vems = make_pair_setup(0)
            for em in ems:
                em()
            load_deferred()
            states = {0: st}
            tail = deque(vems)
            for p in range(NPAIR):
                if p + 1 < NPAIR:
                    states[p + 1], nxt, nxtv = make_pair_setup(p + 1)
                    nxt = deque(nxt + nxtv)
                else:
                    nxt = deque()
                for hh in range(2):
                    yht = yhp.tile([64, K], BF16, tag="yh", name=f"yh{p}{hh}")
                    for ih in range(NH):
                        fillers = deque(tail)
                        if hh == 0:
                            # spread next-pair setup over this head's halves
                            take = (len(nxt) + 1) // 2 if ih == 0 else len(nxt)
                            for _ in range(take):
                                fillers.append(nxt.popleft())
                        tail = emit_attention_half(p, hh, ih, states[p],
                                                   fillers, yht)
                        for em in fillers:
                            em()
            for em in tail:
                em()

            # ---- output projection (partial over this core's 384 dy)
            for tt in range(NT):
                tsl = slice(tt * 128, (tt + 1) * 128)
                ot = osp.tile([128, D], F32, tag="ost", name=f"ot{tt}")
                for oc, osz in ((0, 512), (1, 256)):
                    ps = ps_a.tile([128, 1024], F32, tag="a", name=f"po{tt}{oc}")
                    for c in range(NPAIR):
                        nc.tensor.matmul(
                            ps[:, 0:osz],
                            ypk[c][:, tsl],
                            wos[c][:, oc * 512:oc * 512 + osz],
                            start=(c == 0), stop=(c == NPAIR - 1),
                        )
                    nc.vector.tensor_copy(ot[:, oc * 512:oc * 512 + osz],
                                          ps[:, 0:osz])
                nc.sync.dma_start(out=out[tsl, :], in_=ot)

    if split_waits:
        _split_multi_waits(nc)
    return nc


_NC = {}


def _get_nc(skip=True):
    if skip not in _NC:
        _NC[skip] = _build_nc(skip)
    return _NC[skip]


def sort_ok(is_context):
    n_ctx = np.asarray(is_context).sum(axis=1)
    return bool(np.all((n_ctx >= 896) & (n_ctx <= 128 * 9)))


def make_in_maps(x, is_context, Wq, bq, Wk, bk, Wv, bv, Wo, sort=True):
    """Host-side sharding/layout prep: per-core input dicts.

    With sort=True tokens are reordered context-first per batch (enables the
    masked-block skip); returns (in_maps, perms) where perms[b] un-sorts the
    output rows.
    """
    in_maps = []
    xTb = {}
    uvb = {}
    wvb = {}
    perms = []
    for b in range(B):
        ctx = is_context[b].astype(np.float32)
        if sort:
            perm = np.argsort(~is_context[b], kind="stable")
        else:
            perm = np.arange(K)
        perms.append(perm)
        xb = x[b][perm]
        ctx = ctx[perm]
        xTb[b] = np.ascontiguousarray(xb.T).astype(bf16).reshape(NCHUNK, 128, K)
        uvb[b] = ctx.reshape(1, K).astype(bf16)
        wvb[b] = (NEG * (1.0 - ctx)).reshape(1, K).astype(bf16)
    WoT = np.ascontiguousarray(Wo.T)  # [dy, dout]
    for core in range(N_CORES):
        b = core // 2
        half = core % 2
        sel = slice(384 * half, 384 * (half + 1))
        wvT_s = Wv[sel].T.astype(np.float32)  # [768, 384]
        wvTe = np.zeros((768, 390), np.float32)
        bve = np.zeros(390, np.float32)
        for pp in range(NPAIR):
            for hh in range(2):
                base = pp * 130 + hh * 65
                wcol = pp * 128 + hh * 64
                wvTe[:, base:base + 64] = wvT_s[:, wcol:wcol + 64]
                bve[base:base + 64] = bv[sel][wcol:wcol + 64]
                bve[base + 64] = 1.0
        in_maps.append({
            "xT": xTb[b],
            "wqT": np.ascontiguousarray(Wq[sel].T).astype(bf16).reshape(NCHUNK, 128, 384),
            "wkT": np.ascontiguousarray(Wk[sel].T).astype(bf16).reshape(NCHUNK, 128, 384),
            "wvT": wvTe.astype(bf16).reshape(NCHUNK, 128, 390),
            "woT": WoT[sel].astype(bf16).reshape(NPAIR, 128, D),
            "bqv": np.ascontiguousarray(bq[sel].reshape(NPAIR, 128).T).astype(np.float32),
            "bkv": np.ascontiguousarray(bk[sel].reshape(NPAIR, 128).T).astype(np.float32),
            "bvv": bve.astype(bf16).reshape(1, 390),
            "uv": uvb[b],
            "wv": wvb[b],
            "idm": np.eye(128, dtype=np.float32),
        })
    return in_maps, perms


def combine_results(results, bo, perms):
    out = np.zeros((B, K, D), np.float32)
    for b in range(B):
        out[b][perms[b]] = (
            results[2 * b]["out"] + results[2 * b + 1]["out"] + bo
        )
    return out


def kernel(x, is_context, Wq, bq, Wk, bk, Wv, bv, Wo, bo):
    x = np.asarray(x, np.float32)
    is_context = np.asarray(is_context)
    args = [np.asarray(a, np.float32) for a in (Wq, bq, Wk, bk, Wv, bv, Wo)]
    sort = sort_ok(is_context)
    nc = _get_nc(sort)
    in_maps, perms = make_in_maps(x, is_context, *args, sort=sort)
    res = run_bass_kernel_spmd(nc, in_maps, list(range(N_CORES)), trace=False)
    return combine_results(res.results, np.asarray(bo, np.float32), perms)
